# revision 2
# baseline (speedup 1.0000x reference)
"""Trainium2 Bass kernel for nn_EntityCell — fp16 I/O + baseline-proven ops.

Host staging (layout/dtype only): inputs cast fp16; keys/enc pre-transposed
d-major per chunk; prev stays c-major (needed for update) and is PE-transposed
on chip. Output stored fp16 c-major, host upcasts to fp32.

Per-core, per 128-row chunk (8 chunks):
  DMA:  h16 [c,(e d)], kT [d,(e c)], sT [d,c] fp16 in; o16 fp16 out.
  PE:   h transposes (PSUM, Act evac); gates per-entity reduce; main
        per-entity matmuls hT_e@U + kT_e@V + sT@W (fp16 in, fp32 acc).
  Pool: hkT = hT + kT; t2T = hkT * sT_bcast.
  Act:  evac, sigmoid, tanh.
  DVE:  per-entity stt update, bn_stats norm, Newton rsqrt, fp16 scale.
"""

import numpy as np
from contextlib import nullcontext as _nullctx

B, E, D = 8192, 20, 128
N_CORES = 8
B_LOC = B // N_CORES
CHUNK = 128
NCH = B_LOC // CHUNK
EG = 4

_CACHE = {}


def _build_nc(reps=1, loop_n=None, ablate='full', io_bufs=3, tr_bufs=2,
              bf_bufs=2, sm_bufs=4, psk_bufs=3, psm_bufs=4, psg_bufs=1,
              h_evac='act', newton_iters=2, scale_fp16=True):
    import concourse.tile as tile
    from concourse import bacc, mybir
    from concourse.masks import make_identity
    from contextlib import ExitStack

    fp32 = mybir.dt.float32
    fp16 = mybir.dt.float16
    int32 = mybir.dt.int32
    AF = mybir.ActivationFunctionType
    OP = mybir.AluOpType

    nc = bacc.Bacc("TRN2", target_bir_lowering=False, debug=False)
    h_d = nc.declare_dram_parameter("h", [B_LOC, E, D], fp16, isOutput=False)
    kt_d = nc.declare_dram_parameter("kT", [NCH, D, E, CHUNK], fp16, isOutput=False)
    st_d = nc.declare_dram_parameter("sT", [NCH, D, CHUNK], fp16, isOutput=False)
    u_d = nc.declare_dram_parameter("U", [D, D], fp16, isOutput=False)
    v_d = nc.declare_dram_parameter("V", [D, D], fp16, isOutput=False)
    w_d = nc.declare_dram_parameter("W", [D, D], fp16, isOutput=False)
    out_d = nc.declare_dram_parameter("out", [B_LOC, E, D], fp16, isOutput=True)

    h_v = h_d[:].rearrange("(n p) e d -> n p (e d)", p=CHUNK)
    kt_v = kt_d[:].rearrange("n d e c -> n d (e c)")
    out_v = out_d[:].rearrange("(n p) e d -> n p (e d)", p=CHUNK)

    with ExitStack() as ctx:
        tc = ctx.enter_context(tile.TileContext(nc))
        const_pool = ctx.enter_context(tc.tile_pool(name="const", bufs=1))
        io_pool = ctx.enter_context(tc.tile_pool(name="io", bufs=io_bufs))
        tr_pool = ctx.enter_context(tc.tile_pool(name="tr", bufs=tr_bufs))
        bf_pool = ctx.enter_context(tc.tile_pool(name="bf", bufs=bf_bufs))
        sm_pool = ctx.enter_context(tc.tile_pool(name="sm", bufs=sm_bufs))
        psk_pool = ctx.enter_context(tc.tile_pool(name="psk", bufs=psk_bufs, space="PSUM"))
        psm_pool = ctx.enter_context(tc.tile_pool(name="psm", bufs=psm_bufs, space="PSUM"))
        psg_pool = ctx.enter_context(tc.tile_pool(name="psg", bufs=psg_bufs, space="PSUM"))

        u16c = const_pool.tile([D, D], fp16)
        v16c = const_pool.tile([D, D], fp16)
        w16c = const_pool.tile([D, D], fp16)
        nc.sync.dma_start(u16c[:], u_d[:])
        nc.sync.dma_start(v16c[:], v_d[:])
        nc.sync.dma_start(w16c[:], w_d[:])
        ones16 = const_pool.tile([D, 1], fp16)
        nc.gpsimd.memset(ones16[:], 1.0)
        ident16 = const_pool.tile([D, D], fp16)
        make_identity(nc, ident16[:])
        magic = const_pool.tile([CHUNK, E], int32)
        nc.gpsimd.memset(magic[:], 0x5F3759DF)

        loop_cm = (
            tc.For_i(0, loop_n, 1, hint_engines=tuple(mybir.ALL_ENGINES))
            if loop_n is not None
            else _nullctx()
        )
        with loop_cm:
         for cn in range(NCH * reps):
            n = cn % NCH
            h16 = io_pool.tile([CHUNK, E, D], fp16, name="h16")
            nc.sync.dma_start(h16[:].rearrange("p e d -> p (e d)"), h_v[n])
            kT = io_pool.tile([D, E, CHUNK], fp16, name="kT")
            nc.sync.dma_start(kT[:].rearrange("d e c -> d (e c)"), kt_v[n])
            sT = io_pool.tile([D, CHUNK], fp16, name="sT")
            nc.sync.dma_start(sT[:], st_d[n])

            if ablate == 'dma':
                nc.sync.dma_start(out=out_v[n],
                                  in_=h16[:].rearrange("p e d -> p (e d)"))
                continue

            # ---- per group: hT transposes then main matmuls immediately ----
            # (gates matmuls issue last so the in-order PE never stalls on
            # Pool's t2T; Pool passes overlap the main matmul stream)
            hT = tr_pool.tile([D, E, CHUNK], fp16, name="hT")
            ht16 = bf_pool.tile([CHUNK, E, D], fp16, name="ht16")
            hkT = tr_pool.tile([D, E, CHUNK], fp16, name="hkT")
            sTb = sT[:].unsqueeze(1).broadcast_to([D, E, CHUNK])
            t2T = hkT
            for gi in range(E // EG):
                sl = slice(gi * EG, (gi + 1) * EG)
                htp = psk_pool.tile([D, EG, CHUNK], fp16, name="htp")
                for j in range(EG):
                    nc.tensor.transpose(htp[:, j], h16[:, gi * EG + j], ident16[:])
                if h_evac == 'act':
                    nc.scalar.copy(hT[:, sl], htp[:])
                else:
                    nc.vector.tensor_copy(hT[:, sl], htp[:])
                ps = psm_pool.tile([CHUNK, EG, D], fp32, name="ps")
                for j in range(EG):
                    e = gi * EG + j
                    nc.tensor.matmul(ps[:, j], hT[:, e], u16c[:], start=True, stop=False)
                    nc.tensor.matmul(ps[:, j], kT[:, e], v16c[:], start=False, stop=False)
                    nc.tensor.matmul(ps[:, j], sT[:], w16c[:], start=False, stop=True)
                nc.scalar.activation(ht16[:, sl], ps[:], AF.Tanh)
                nc.gpsimd.tensor_tensor(hkT[:, sl], hT[:, sl], kT[:, sl], OP.add)
                nc.gpsimd.tensor_tensor(t2T[:, sl], hkT[:, sl], sTb[:, sl], OP.mult)

            # ---- gates reduce + sigmoid ----
            gps = psg_pool.tile([CHUNK, E], fp32, name="gps")
            for e in range(E):
                nc.tensor.matmul(gps[:, e:e + 1], t2T[:, e], ones16[:],
                                 start=True, stop=True)
            g32 = sm_pool.tile([CHUNK, E], fp32, name="g32")
            nc.scalar.activation(g32[:], gps[:], AF.Sigmoid)

            if ablate == 'compute':
                nc.sync.dma_start(out=out_v[n],
                                  in_=ht16[:].rearrange("p e d -> p (e d)"))
                continue

            # ---- update u = g*h_tilda + h (per-entity stt, in place) ----
            u16 = ht16
            for e in range(E):
                nc.vector.scalar_tensor_tensor(
                    u16[:, e], ht16[:, e], g32[:, e:e + 1], h16[:, e],
                    OP.mult, OP.add)

            # ---- sum(u^2) via bn_stats ----
            bn = sm_pool.tile([CHUNK, E, 6], fp32, name="bn")
            for e in range(E):
                nc.vector.bn_stats(bn[:, e, :], u16[:, e])
            t_a = sm_pool.tile([CHUNK, E], fp32, name="t_a")
            nc.vector.tensor_tensor(t_a[:], bn[:, :, 1], bn[:, :, 1], OP.mult)
            t_b = sm_pool.tile([CHUNK, E], fp32, name="t_b")
            nc.vector.tensor_tensor(t_b[:], bn[:, :, 4], bn[:, :, 4], OP.mult)
            t_ab = sm_pool.tile([CHUNK, E], fp32, name="t_ab")
            nc.vector.tensor_tensor(t_ab[:], t_a[:], t_b[:], OP.add)
            t_c = sm_pool.tile([CHUNK, E], fp32, name="t_c")
            nc.vector.tensor_tensor(t_c[:], bn[:, :, 2], bn[:, :, 5], OP.add)
            a32 = sm_pool.tile([CHUNK, E], fp32, name="a32")
            nc.vector.scalar_tensor_tensor(a32[:], t_ab[:], 64.0, t_c[:],
                                           OP.mult, OP.add)
            nc.vector.tensor_scalar(a32[:], a32[:], 1e-12, None, op0=OP.max)

            # ---- Newton rsqrt ----
            ti = sm_pool.tile([CHUNK, E], int32, name="ti")
            nc.vector.tensor_scalar(ti[:], a32[:].bitcast(int32), 1, None,
                                    op0=OP.logical_shift_right)
            yi = sm_pool.tile([CHUNK, E], int32, name="yi")
            nc.vector.tensor_tensor(yi[:], magic[:], ti[:], OP.subtract)
            y = yi[:].bitcast(fp32)
            for _ in range(newton_iters):
                y2 = sm_pool.tile([CHUNK, E], fp32, name="y2")
                nc.vector.tensor_tensor(y2[:], y, y, OP.mult)
                tt = sm_pool.tile([CHUNK, E], fp32, name="tt")
                nc.vector.tensor_tensor(tt[:], a32[:], y2[:], OP.mult)
                ww = sm_pool.tile([CHUNK, E], fp32, name="ww")
                nc.vector.tensor_scalar(ww[:], tt[:], -0.5, 1.5,
                                        op0=OP.mult, op1=OP.add)
                yn = sm_pool.tile([CHUNK, E], fp32, name="yn")
                nc.vector.tensor_tensor(yn[:], y, ww[:], OP.mult)
                y = yn[:]

            # ---- scale + store (fp16) ----
            o16 = io_pool.tile([CHUNK, E, D], fp16, name="o16")
            for e in range(E):
                nc.vector.tensor_scalar(o16[:, e], u16[:, e], y[:, e:e + 1],
                                        None, op0=OP.mult)
            nc.sync.dma_start(out=out_v[n], in_=o16[:].rearrange("p e d -> p (e d)"))

    nc.compile()
    return nc


def make_in_maps(encoded_sents, prev_states, keys, U, V, W):
    """Host staging: fp16 casts + d-major pre-transposes of keys/enc."""
    enc = np.asarray(encoded_sents, dtype=np.float16)
    prev = np.asarray(prev_states, dtype=np.float16)
    kys = np.asarray(keys, dtype=np.float16)
    U16 = np.ascontiguousarray(np.asarray(U, dtype=np.float16))
    V16 = np.ascontiguousarray(np.asarray(V, dtype=np.float16))
    W16 = np.ascontiguousarray(np.asarray(W, dtype=np.float16))

    in_maps = []
    for i in range(N_CORES):
        lo, hi = i * B_LOC, (i + 1) * B_LOC
        kt = np.ascontiguousarray(
            kys[lo:hi].reshape(NCH, CHUNK, E, D).transpose(0, 3, 2, 1))
        st = np.ascontiguousarray(
            enc[lo:hi].reshape(NCH, CHUNK, D).transpose(0, 2, 1))
        in_maps.append({
            "h": np.ascontiguousarray(prev[lo:hi]),
            "kT": kt,
            "sT": st,
            "U": U16, "V": V16, "W": W16,
        })
    return in_maps


def _get_nc():
    if "nc" not in _CACHE:
        _CACHE["nc"] = _build_nc()
    return _CACHE["nc"]


def kernel(encoded_sents, prev_states, keys, U, V, W):
    import sys

    if "/opt/trn_rl_repo" not in sys.path:
        sys.path.insert(0, "/opt/trn_rl_repo")
    from concourse.bass_utils import run_bass_kernel_spmd

    nc = _get_nc()
    in_maps = make_in_maps(encoded_sents, prev_states, keys, U, V, W)
    res = run_bass_kernel_spmd(nc, in_maps, list(range(N_CORES)))
    out = np.concatenate([res.results[i]["out"] for i in range(N_CORES)], axis=0)
    return out.astype(np.float32)


# revision 5
# speedup vs baseline: 1.0118x; 1.0118x over previous
"""Trainium2 Bass kernel for nn_EntityCell — fp16 I/O + baseline-proven ops.

Host staging (layout/dtype only): inputs cast fp16; keys/enc pre-transposed
d-major per chunk; prev stays c-major (needed for update) and is PE-transposed
on chip. Output stored fp16 c-major, host upcasts to fp32.

Per-core, per 128-row chunk (8 chunks):
  DMA:  h16 [c,(e d)], kT [d,(e c)], sT [d,c] fp16 in; o16 fp16 out.
  PE:   h transposes (PSUM, Act evac); gates per-entity reduce; main
        per-entity matmuls hT_e@U + kT_e@V + sT@W (fp16 in, fp32 acc).
  Pool: hkT = hT + kT; t2T = hkT * sT_bcast.
  Act:  evac, sigmoid, tanh.
  DVE:  per-entity stt update, bn_stats norm, Newton rsqrt, fp16 scale.
"""

import numpy as np
from contextlib import nullcontext as _nullctx

B, E, D = 8192, 20, 128
N_CORES = 8
B_LOC = B // N_CORES
CHUNK = 128
NCH = B_LOC // CHUNK
EG = 4

_CACHE = {}


def _build_nc(reps=1, loop_n=None, ablate='full', io_bufs=3, tr_bufs=2,
              bf_bufs=2, sm_bufs=4, psk_bufs=3, psm_bufs=4, psg_bufs=1,
              h_evac='act', newton_iters=2, upd_pool=0, scale_pool=0):
    import concourse.tile as tile
    from concourse import bacc, mybir
    from concourse.masks import make_identity
    from contextlib import ExitStack

    fp32 = mybir.dt.float32
    fp16 = mybir.dt.float16
    int32 = mybir.dt.int32
    AF = mybir.ActivationFunctionType
    OP = mybir.AluOpType

    nc = bacc.Bacc("TRN2", target_bir_lowering=False, debug=False)
    h_d = nc.declare_dram_parameter("h", [B_LOC, E, D], fp16, isOutput=False)
    kt_d = nc.declare_dram_parameter("kT", [NCH, D, E, CHUNK], fp16, isOutput=False)
    st_d = nc.declare_dram_parameter("sT", [NCH, D, CHUNK], fp16, isOutput=False)
    u_d = nc.declare_dram_parameter("U", [D, D], fp16, isOutput=False)
    v_d = nc.declare_dram_parameter("V", [D, D], fp16, isOutput=False)
    w_d = nc.declare_dram_parameter("W", [D, D], fp16, isOutput=False)
    out_d = nc.declare_dram_parameter("out", [B_LOC, E, D], fp16, isOutput=True)

    h_v = h_d[:].rearrange("(n p) e d -> n p (e d)", p=CHUNK)
    kt_v = kt_d[:].rearrange("n d e c -> n d (e c)")
    out_v = out_d[:].rearrange("(n p) e d -> n p (e d)", p=CHUNK)

    with ExitStack() as ctx:
        tc = ctx.enter_context(tile.TileContext(nc))
        const_pool = ctx.enter_context(tc.tile_pool(name="const", bufs=1))
        io_pool = ctx.enter_context(tc.tile_pool(name="io", bufs=io_bufs))
        tr_pool = ctx.enter_context(tc.tile_pool(name="tr", bufs=tr_bufs))
        bf_pool = ctx.enter_context(tc.tile_pool(name="bf", bufs=bf_bufs))
        sm_pool = ctx.enter_context(tc.tile_pool(name="sm", bufs=sm_bufs))
        psk_pool = ctx.enter_context(tc.tile_pool(name="psk", bufs=psk_bufs, space="PSUM"))
        psm_pool = ctx.enter_context(tc.tile_pool(name="psm", bufs=psm_bufs, space="PSUM"))
        psg_pool = ctx.enter_context(tc.tile_pool(name="psg", bufs=psg_bufs, space="PSUM"))

        u16c = const_pool.tile([D, D], fp16)
        v16c = const_pool.tile([D, D], fp16)
        w16c = const_pool.tile([D, D], fp16)
        nc.sync.dma_start(u16c[:], u_d[:])
        nc.sync.dma_start(v16c[:], v_d[:])
        nc.sync.dma_start(w16c[:], w_d[:])
        ones16 = const_pool.tile([D, 1], fp16)
        nc.gpsimd.memset(ones16[:], 1.0)
        ident16 = const_pool.tile([D, D], fp16)
        make_identity(nc, ident16[:])
        magic = const_pool.tile([CHUNK, E], int32)
        nc.gpsimd.memset(magic[:], 0x5F3759DF)

        loop_cm = (
            tc.For_i(0, loop_n, 1, hint_engines=tuple(mybir.ALL_ENGINES))
            if loop_n is not None
            else _nullctx()
        )
        with loop_cm:
         for cn in range(NCH * reps):
            n = cn % NCH
            h16 = io_pool.tile([CHUNK, E, D], fp16, name="h16")
            nc.sync.dma_start(h16[:].rearrange("p e d -> p (e d)"), h_v[n])
            kT = io_pool.tile([D, E, CHUNK], fp16, name="kT")
            nc.sync.dma_start(kT[:].rearrange("d e c -> d (e c)"), kt_v[n])
            sT = io_pool.tile([D, CHUNK], fp16, name="sT")
            nc.sync.dma_start(sT[:], st_d[n])

            if ablate == 'dma':
                nc.sync.dma_start(out=out_v[n],
                                  in_=h16[:].rearrange("p e d -> p (e d)"))
                continue

            # ---- per group: hT transposes then main matmuls immediately ----
            # (gates matmuls issue last so the in-order PE never stalls on
            # Pool's t2T; Pool passes overlap the main matmul stream)
            hT = tr_pool.tile([D, E, CHUNK], fp16, name="hT")
            ht16 = bf_pool.tile([CHUNK, E, D], fp16, name="ht16")
            hkT = tr_pool.tile([D, E, CHUNK], fp16, name="hkT")
            sTb = sT[:].unsqueeze(1).broadcast_to([D, E, CHUNK])
            t2T = hkT
            for gi in range(E // EG):
                sl = slice(gi * EG, (gi + 1) * EG)
                htp = psk_pool.tile([D, EG, CHUNK], fp16, name="htp")
                for j in range(EG):
                    nc.tensor.transpose(htp[:, j], h16[:, gi * EG + j], ident16[:])
                if h_evac == 'act':
                    nc.scalar.copy(hT[:, sl], htp[:])
                else:
                    nc.vector.tensor_copy(hT[:, sl], htp[:])
                ps = psm_pool.tile([CHUNK, EG, D], fp32, name="ps")
                for j in range(EG):
                    e = gi * EG + j
                    nc.tensor.matmul(ps[:, j], hT[:, e], u16c[:], start=True, stop=False)
                    nc.tensor.matmul(ps[:, j], kT[:, e], v16c[:], start=False, stop=False)
                    nc.tensor.matmul(ps[:, j], sT[:], w16c[:], start=False, stop=True)
                nc.scalar.activation(ht16[:, sl], ps[:], AF.Tanh)
                nc.gpsimd.tensor_tensor(hkT[:, sl], hT[:, sl], kT[:, sl], OP.add)
                nc.gpsimd.tensor_tensor(t2T[:, sl], hkT[:, sl], sTb[:, sl], OP.mult)

            # ---- gates reduce + sigmoid ----
            gps = psg_pool.tile([CHUNK, E], fp32, name="gps")
            for e in range(E):
                nc.tensor.matmul(gps[:, e:e + 1], t2T[:, e], ones16[:],
                                 start=True, stop=True)
            g32 = sm_pool.tile([CHUNK, E], fp32, name="g32")
            nc.scalar.activation(g32[:], gps[:], AF.Sigmoid)

            if ablate == 'compute':
                nc.sync.dma_start(out=out_v[n],
                                  in_=ht16[:].rearrange("p e d -> p (e d)"))
                continue

            # ---- update u = g*h_tilda + h (per-entity stt, in place) ----
            u16 = ht16
            for e in range(E):
                eng = nc.gpsimd if e < upd_pool else nc.vector
                eng.scalar_tensor_tensor(
                    u16[:, e], ht16[:, e], g32[:, e:e + 1], h16[:, e],
                    OP.mult, OP.add)

            # ---- sum(u^2) via bn_stats ----
            bn = sm_pool.tile([CHUNK, E, 6], fp32, name="bn")
            for e in range(E):
                nc.vector.bn_stats(bn[:, e, :], u16[:, e])
            t_a = sm_pool.tile([CHUNK, E], fp32, name="t_a")
            nc.vector.tensor_tensor(t_a[:], bn[:, :, 1], bn[:, :, 1], OP.mult)
            t_b = sm_pool.tile([CHUNK, E], fp32, name="t_b")
            nc.vector.tensor_tensor(t_b[:], bn[:, :, 4], bn[:, :, 4], OP.mult)
            t_ab = sm_pool.tile([CHUNK, E], fp32, name="t_ab")
            nc.vector.tensor_tensor(t_ab[:], t_a[:], t_b[:], OP.add)
            t_c = sm_pool.tile([CHUNK, E], fp32, name="t_c")
            nc.vector.tensor_tensor(t_c[:], bn[:, :, 2], bn[:, :, 5], OP.add)
            a32 = sm_pool.tile([CHUNK, E], fp32, name="a32")
            nc.vector.scalar_tensor_tensor(a32[:], t_ab[:], 64.0, t_c[:],
                                           OP.mult, OP.add)
            nc.vector.tensor_scalar(a32[:], a32[:], 1e-12, None, op0=OP.max)

            # ---- Newton rsqrt ----
            ti = sm_pool.tile([CHUNK, E], int32, name="ti")
            nc.vector.tensor_scalar(ti[:], a32[:].bitcast(int32), 1, None,
                                    op0=OP.logical_shift_right)
            yi = sm_pool.tile([CHUNK, E], int32, name="yi")
            nc.vector.tensor_tensor(yi[:], magic[:], ti[:], OP.subtract)
            y = yi[:].bitcast(fp32)
            for _ in range(newton_iters):
                y2 = sm_pool.tile([CHUNK, E], fp32, name="y2")
                nc.vector.tensor_tensor(y2[:], y, y, OP.mult)
                tt = sm_pool.tile([CHUNK, E], fp32, name="tt")
                nc.vector.tensor_tensor(tt[:], a32[:], y2[:], OP.mult)
                ww = sm_pool.tile([CHUNK, E], fp32, name="ww")
                nc.vector.tensor_scalar(ww[:], tt[:], -0.5, 1.5,
                                        op0=OP.mult, op1=OP.add)
                yn = sm_pool.tile([CHUNK, E], fp32, name="yn")
                nc.vector.tensor_tensor(yn[:], y, ww[:], OP.mult)
                y = yn[:]

            # ---- scale + store (fp16) ----
            o16 = io_pool.tile([CHUNK, E, D], fp16, name="o16")
            for e in range(E):
                eng = nc.gpsimd if e < scale_pool else nc.vector
                eng.tensor_scalar(o16[:, e], u16[:, e], y[:, e:e + 1],
                                  None, op0=OP.mult)
            nc.sync.dma_start(out=out_v[n], in_=o16[:].rearrange("p e d -> p (e d)"))

    nc.compile()
    return nc


def make_in_maps(encoded_sents, prev_states, keys, U, V, W):
    """Host staging: fp16 casts + d-major pre-transposes of keys/enc."""
    enc = np.asarray(encoded_sents, dtype=np.float16)
    prev = np.asarray(prev_states, dtype=np.float16)
    kys = np.asarray(keys, dtype=np.float16)
    U16 = np.ascontiguousarray(np.asarray(U, dtype=np.float16))
    V16 = np.ascontiguousarray(np.asarray(V, dtype=np.float16))
    W16 = np.ascontiguousarray(np.asarray(W, dtype=np.float16))

    in_maps = []
    for i in range(N_CORES):
        lo, hi = i * B_LOC, (i + 1) * B_LOC
        kt = np.ascontiguousarray(
            kys[lo:hi].reshape(NCH, CHUNK, E, D).transpose(0, 3, 2, 1))
        st = np.ascontiguousarray(
            enc[lo:hi].reshape(NCH, CHUNK, D).transpose(0, 2, 1))
        in_maps.append({
            "h": np.ascontiguousarray(prev[lo:hi]),
            "kT": kt,
            "sT": st,
            "U": U16, "V": V16, "W": W16,
        })
    return in_maps


def _get_nc():
    if "nc" not in _CACHE:
        _CACHE["nc"] = _build_nc()
    return _CACHE["nc"]


def kernel(encoded_sents, prev_states, keys, U, V, W):
    import sys

    if "/opt/trn_rl_repo" not in sys.path:
        sys.path.insert(0, "/opt/trn_rl_repo")
    from concourse.bass_utils import run_bass_kernel_spmd

    nc = _get_nc()
    in_maps = make_in_maps(encoded_sents, prev_states, keys, U, V, W)
    res = run_bass_kernel_spmd(nc, in_maps, list(range(N_CORES)))
    out = np.concatenate([res.results[i]["out"] for i in range(N_CORES)], axis=0)
    return out.astype(np.float32)
